# revision 55
# baseline (speedup 1.0000x reference)
"""Trainium2 Bass kernel for nn_Attention_54391465836966.

Math (per batch b):
  ctok = content_feat[b].raw_reshape(S,C) + pos         # [1024, 512]
  comp_tok[n] = components[n,b].raw_reshape(S,C) + pos
  q = ctok @ Wq ; k[n],v[n] = comp_tok[n] @ Wkv (split)
  per head h, comp n: P = exp(scale * q_h k_h^T); o_nh = (P @ v_nh) / rowsum(P)
  result = sum_n o_n ; s = (result + ctok) @ Wproj + bproj
  out = Wconv^T[C:] @ cf2d + Wconv^T[:C] @ s2d + bconv
    (s2d = raw reshape of the token-major s buffer; cf2d = content_feat[b])

Sharding: 8 cores <- (b, n) pairs; b = core//4, n = core%4.  Everything after
`result` is affine in the component partial, so each core applies the linear
tail to its own o_n (constant terms gated to the n==0 core via zeroed
per-core inputs) and the host sums the four partial outputs per batch.  The
cf half of the conv depends only on b, so it is additionally sharded over n:
each core computes just its own 128-row output-channel chunk (out_cf) and
the host scatter-adds it into the gathered result.

Implementation notes:
- The reference's token matrices are RAW reshapes of [C,H,W] buffers (the
  same even/odd interleave as the final s->s2d reshape), so the host ships
  (raw_reshape(x) + pos).T directly (bf16); nothing on-chip ever transposes
  tokens.  Wconv is host-transposed as well.
- All matmul operands are bf16 (fp32 PSUM accumulate).  DMA volume is half
  of fp32, and transposes run at 1.0 cycles/row.
- Attention is ACT-bound (64 exps of [128,1024] ~= 1 us each), so the
  kT/qT matmuls of later head-pairs are interleaved INTO the attention
  stream to soak up spare PE cycles.  PSUM budget makes this possible:
  o is single-buffered and drained immediately with UNnormalized copies;
  normalization (reciprocal_approx_fast + partition_broadcast + one
  pair-wide [128,S] multiply) happens in SBUF afterwards
  -> mm(2) + sc(2x2) + o(2) = 8 banks.  Each input tensor is a single
  merged DMA (per-DMA cost is ~650ns serial SP.SEQ + ~625ns HWDGE + 900ns
  sem latency), and dep-free warmup transposes spin the PE clock up past
  its p-state ramp before the first real matmuls.
- Projection is computed transposed (stT = Wproj^T @ s_in^T) with head
  pairs packed into 128-partition tiles (full-K matmuls), bias via
  per-partition tensor_scalar on the PSUM drain.  The s->s2d raw-reshape
  is 32 PE transposes of stride-2 column slices, interleaved parity-major
  with the conv so output DMAs start early.  The cf half of the conv is
  emitted at the head of the tail to cover the last pair's normalization
  chain.
"""
import sys

sys.path.insert(0, "/opt/trn_rl_repo")

import numpy as np

N_CORES = 8
B, C, H, W = 2, 512, 32, 32
S = H * W  # 1024
NH, HD = 8, 64
SCALE = HD ** -0.5

_CACHE = {}


def _build():
    if "nc" in _CACHE:
        return _CACHE["nc"]
    from contextlib import ExitStack

    import concourse.bacc as bacc
    import concourse.mybir as mybir
    import concourse.tile as tile
    from concourse.masks import make_identity

    f32 = mybir.dt.float32
    f32r = mybir.dt.float32r
    bf16 = mybir.dt.bfloat16
    EXP = mybir.ActivationFunctionType.Exp
    IDENT = mybir.ActivationFunctionType.Identity
    MULT = mybir.AluOpType.mult
    ADD = mybir.AluOpType.add

    nc = bacc.Bacc("TRN2", target_bir_lowering=False, debug=False,
                   num_devices=N_CORES)

    din = lambda n, s, dt: nc.dram_tensor(n, s, dt, kind="ExternalInput").ap()
    cf = din("cf", [C, S], bf16)         # content_feat[b] raw [C,S] (conv only)
    ctokTd = din("ctokT", [C, S], bf16)  # (content_tok + pos).T, host-prepped
    compTd = din("compT", [C, S], bf16)  # (comp_tok + pos).T, host-prepped
    wq = din("wq", [C, C], bf16)
    wkv = din("wkv", [C, 2 * C], bf16)   # cols 0:C -> K, C:2C -> V
    wproj = din("wproj", [C, C], bf16)
    wconvT = din("wconvT", [C, C], bf16)      # Wconv.T rows :C (s half)
    wccsel = din("wccsel", [C, 128], bf16)    # Wconv.T[C:, 128n:128n+128]
    bprojT = din("bprojT", [C, 1], f32)       # zeroed n>0
    bconvT = din("bconvT", [C, 1], f32)       # zeroed n>0
    gate = din("gate", [128, 1], f32)         # 1.0 on n==0 cores else 0.0
    out_p = nc.dram_tensor("out_p", [C, S], f32, kind="ExternalOutput").ap()
    out_cf = nc.dram_tensor("out_cf", [128, S], f32,
                            kind="ExternalOutput").ap()

    with tile.TileContext(nc) as tc, ExitStack() as ctx:
        main = ctx.enter_context(tc.tile_pool(name="main", bufs=1))

        # ---- constants ----
        ident32 = main.tile([128, 128], f32, tag="id32")
        make_identity(nc, ident32[:])
        ident = main.tile([128, 128], bf16, tag="ident")
        nc.vector.tensor_copy(ident[:], ident32[:])
        ones_bf = main.tile([128, 8], bf16, tag="ones")
        nc.gpsimd.memset(ones_bf[:], 1.0)
        g_sb = main.tile([128, 1], f32, tag="g")

        # ---- persistent SBUF tiles (one wide tile per DRAM tensor so each
        # input is a SINGLE DMA: per-DMA cost is ~650ns serial SP.SEQ +
        # ~625ns HWDGE + 900ns sem latency, so fewer/bigger wins) ----
        wkv_all = main.tile([128, 4 * 2 * C], bf16, tag="wkva", name="wkva")
        wq_all = main.tile([128, 4 * C], bf16, tag="wqa", name="wqa")
        comp_all = main.tile([128, 4 * S], bf16, tag="cma", name="cma")
        cf_all = main.tile([128, 4 * S], bf16, tag="cfa", name="cfa")
        ctok_all = main.tile([128, 4 * S], bf16, tag="cta", name="cta")
        wc_all = main.tile([128, 4 * C], bf16, tag="wca", name="wca")
        wcsel_all = main.tile([128, 4 * 128], bf16, tag="wcsa", name="wcsa")
        wp_all = main.tile([128, 4 * C], bf16, tag="wpa", name="wpa")
        bpj_all = main.tile([128, 4], f32, tag="bpja", name="bpja")
        bcv_all = main.tile([128, 4], f32, tag="bcva", name="bcva")
        # chunk views (k-th 128-row block of the [C,*] DRAM tensor)
        wkv_sb = [wkv_all[:, 2 * C * k:2 * C * (k + 1)] for k in range(4)]
        wq_sb = [wq_all[:, C * k:C * (k + 1)] for k in range(4)]
        comp_sb = [comp_all[:, S * k:S * (k + 1)] for k in range(4)]
        cf_sb = [cf_all[:, S * k:S * (k + 1)] for k in range(4)]
        ctokT = [ctok_all[:, S * k:S * (k + 1)] for k in range(4)]
        wcs_sb = [wc_all[:, C * k:C * (k + 1)] for k in range(4)]
        wcc_sb = [wcsel_all[:, 128 * k:128 * (k + 1)] for k in range(4)]
        wp_sb = [wp_all[:, C * k:C * (k + 1)] for k in range(4)]
        bcv_sb = [bcv_all[:, i:i + 1] for i in range(4)]
        bpj_sb = [bpj_all[:, i:i + 1] for i in range(4)]
        kT = [main.tile([128, S], bf16, tag=f"kt{j}", name=f"kt{j}")
              for j in range(4)]
        qT = [main.tile([128, S], bf16, tag=f"qt{j}", name=f"qt{j}")
              for j in range(4)]
        v_sb = [main.tile([128, 8 * 65], bf16, tag=f"v{t}", name=f"v{t}")
                for t in range(8)]
        outcf_sb = main.tile([128, S], f32, tag="ocf", name="ocf")
        # rtb[j]: normalized bf16 head pair (2j, 2j+1), later s_in^T chunk j
        rtb = [main.tile([128, S], bf16, tag=f"rt{j}", name=f"rt{j}")
               for j in range(4)]

        # ---- DMA emission order: attention-critical first ----
        def dma_merged(dst_tile, src_ap, k):
            src3 = src_ap.rearrange("(k p) s -> p k s", k=k)
            dst3 = dst_tile[:].rearrange("p (k s) -> p k s", k=k)
            nc.sync.dma_start(dst3[:, :, :], src3[:, :, :])

        def dma_merged_part(dst_tile, src_ap, k, lo, hi, inner):
            src3 = src_ap.rearrange("(k p) s -> p k s", k=k)
            dst3 = dst_tile[:].rearrange("p (k s) -> p k s", k=k)
            nc.sync.dma_start(dst3[:, lo:hi, 0:inner], src3[:, lo:hi, 0:inner])

        dma_merged_part(comp_all, compTd, 4, 0, 2, S)
        dma_merged_part(wkv_all, wkv, 4, 0, 2, 2 * C)
        dma_merged_part(comp_all, compTd, 4, 2, 4, S)
        dma_merged_part(wkv_all, wkv, 4, 2, 4, 2 * C)
        dma_merged(wq_all, wq, 4)
        dma_merged(ctok_all, ctokTd, 4)
        dma_merged(cf_all, cf, 4)
        dma_merged(wcsel_all, wccsel, 4)
        nc.sync.dma_start(g_sb[:], gate[:])
        dma_merged(bcv_all, bconvT, 4)
        dma_merged(bpj_all, bprojT, 4)
        dma_merged(wp_all, wproj, 4)
        dma_merged(wc_all, wconvT, 4)

        warm_src = main.tile([128, 128], bf16, tag="warm", name="warm")
        nc.gpsimd.memset(warm_src[:], 0.25)

        # one PSUM pool spans startup + attention: mm(1x2) + sc(2x2) + o(2x1)
        # = 8 banks.  The mm tag stays live through attention so the later
        # head-pairs' kT/qT matmuls can interleave into ACT-bound stretches.
        with tc.tile_pool(name="psMain", bufs=1, space="PSUM") as ps:

            def emit_kq(j, dst, wsrc, act, cp_eng):
                for t in range(2):
                    acc = ps.tile([128, 512], f32, tag="mm", bufs=2)
                    for k in range(4):
                        nc.tensor.matmul(acc[:],
                                         wsrc[k][:, 128 * j:128 * (j + 1)],
                                         act[k][:, 512 * t:512 * (t + 1)],
                                         start=(k == 0), stop=(k == 3))
                    cp_eng(dst[j][:, 512 * t:512 * (t + 1)], acc[:])

            for w in range(36):
                wtp = ps.tile([128, 128], bf16, tag="mm", bufs=2)
                nc.tensor.transpose(wtp[:], warm_src[:], ident[:])

            emit_kq(0, kT, wkv_sb, comp_sb, nc.scalar.copy)

            # v (lhsT of the o matmuls).  v0-3 run while the ctok DMA is in
            # flight; qT[0] follows, then v4-7 interleave with head 0's first
            # scores so the ACT exp stream starts as early as possible.
            h0_sc = []

            def emit_v(t):
                nc.vector.tensor_copy(
                    v_sb[t][:].rearrange("p (h e) -> p h e", h=8)[:, :, 64:65],
                    ones_bf[:, 0:8].rearrange("p (h o) -> p h o", o=1))
                acc = ps.tile([128, 512], f32, tag="mm", bufs=2)
                for k in range(4):
                    nc.tensor.matmul(acc[:], comp_sb[k][:, 128 * t:128 * (t + 1)],
                                     wkv_sb[k][:, C:2 * C],
                                     start=(k == 0), stop=(k == 3))
                nc.vector.tensor_copy(
                    v_sb[t][:].rearrange("p (h e) -> p h e", h=8)[:, :, 0:64],
                    acc[:].rearrange("p (h d) -> p h d", h=8))

            def emit_h0_sc(t):
                sc = ps.tile([128, S], f32, tag="sc", bufs=2)
                for qc in range(2):
                    nc.tensor.matmul(
                        sc[:, 512 * qc:512 * (qc + 1)],
                        kT[0][0:64, 128 * t:128 * (t + 1)],
                        qT[0][0:64, 512 * qc:512 * (qc + 1)],
                        start=True, stop=True)
                pt = main.tile([128, S], bf16, tag=f"pt{t}", name=f"pt0_{t}")
                nc.scalar.activation(pt[:], sc[:], EXP, scale=SCALE)
                h0_sc.append((sc, pt))

            for t in range(4):
                emit_v(t)
            emit_kq(0, qT, wq_sb, ctokT, nc.scalar.copy)
            for t in range(4):
                emit_h0_sc(t)
                emit_v(4 + t)
            for t in range(4, 8):
                emit_h0_sc(t)

            # ---- attention ----
            # head order: pairs 0..3; within the LAST pair the odd head goes
            # first so the closing normalization chain is one op shorter.
            heads = [0, 1, 2, 3, 4, 5, 7, 6]
            sc_prev = None
            pre_sc1 = [None]
            pair_state = {}

            def emit_conv_cf(pc, cp_eng, pool=None, tag="mm"):
                acc = (pool or ps).tile([128, 512], f32, tag=tag, bufs=2)
                for k2 in range(4):
                    nc.tensor.matmul(acc[:], wcc_sb[k2][:],
                                     cf_sb[k2][:, 512 * pc:512 * (pc + 1)],
                                     start=(k2 == 0), stop=(k2 == 3))
                cp_eng(outcf_sb[:, 512 * pc:512 * (pc + 1)], acc[:])
                nc.sync.dma_start(
                    out_cf[:, 512 * pc:512 * (pc + 1)],
                    outcf_sb[:, 512 * pc:512 * (pc + 1)])
            for idx, h in enumerate(heads):
                jq, row = h // 2, 64 * (h % 2)
                o_ps = ps.tile([65, S], f32, tag="o", bufs=1)
                scs = []
                for kt in range(8):
                    if idx == 1 and kt == 4:
                        emit_kq(1, kT, wkv_sb, comp_sb, nc.vector.tensor_copy)
                    elif idx == 1 and kt == 5:
                        emit_kq(1, qT, wq_sb, ctokT, nc.vector.tensor_copy)
                    elif kt == 6 and 2 <= idx < 6 and jq < 3:
                        if idx % 2 == 0:
                            emit_kq(jq + 1, kT, wkv_sb, comp_sb,
                                    nc.vector.tensor_copy)
                        else:
                            emit_kq(jq + 1, qT, wq_sb, ctokT,
                                    nc.vector.tensor_copy)
                    if idx == 0:
                        scs.append(None)
                        continue
                    if kt == 1 and idx == 1 and pre_sc1[0] is not None:
                        sc = pre_sc1[0]
                    elif kt > 0:
                        sc = ps.tile([128, S], f32, tag="sc", bufs=2)
                        for qc in range(2):
                            nc.tensor.matmul(
                                sc[:, 512 * qc:512 * (qc + 1)],
                                kT[jq][row:row + 64, 128 * kt:128 * (kt + 1)],
                                qT[jq][row:row + 64, 512 * qc:512 * (qc + 1)],
                                start=True, stop=True)
                    else:
                        sc = sc_prev
                    scs.append(sc)
                for kt in range(8):
                    if idx == 0 and kt in (0, 1):
                        # pre-emit next head's kt=0/1 scores (h0's own scores
                        # were emitted interleaved with v above, so its o
                        # matmuls leave ACT hungry at the boundary)
                        h2 = heads[1]
                        jq2, row2 = h2 // 2, 64 * (h2 % 2)
                        scn = ps.tile([128, S], f32, tag="sc", bufs=2)
                        for qc in range(2):
                            nc.tensor.matmul(
                                scn[:, 512 * qc:512 * (qc + 1)],
                                kT[jq2][row2:row2 + 64,
                                        128 * kt:128 * (kt + 1)],
                                qT[jq2][row2:row2 + 64,
                                        512 * qc:512 * (qc + 1)],
                                start=True, stop=True)
                        if kt == 0:
                            sc_prev = scn
                        else:
                            pre_sc1[0] = scn
                    if idx == 0:
                        pt = h0_sc[kt][1]
                        for qc in range(2):
                            nc.tensor.matmul(
                                o_ps[:, 512 * qc:512 * (qc + 1)],
                                v_sb[kt][:, 65 * h:65 * h + 65],
                                pt[:, 512 * qc:512 * (qc + 1)],
                                start=(kt == 0), stop=(kt == 7))
                        continue
                    if idx < NH - 1 and kt == 7:
                        # pre-emit next head's kt=0 scores (keeps ACT fed
                        # across the head boundary)
                        h2 = heads[idx + 1]
                        jq2, row2 = h2 // 2, 64 * (h2 % 2)
                        sc_prev = ps.tile([128, S], f32, tag="sc", bufs=2)
                        for qc in range(2):
                            nc.tensor.matmul(
                                sc_prev[:, 512 * qc:512 * (qc + 1)],
                                kT[jq2][row2:row2 + 64, 0:128],
                                qT[jq2][row2:row2 + 64,
                                        512 * qc:512 * (qc + 1)],
                                start=True, stop=True)
                    pt = main.tile([128, S], bf16, tag=f"pt{kt}",
                                   name=f"pt{h}_{kt}")
                    nc.scalar.activation(pt[:], scs[kt][:], EXP, scale=SCALE)
                    for qc in range(2):
                        nc.tensor.matmul(
                            o_ps[:, 512 * qc:512 * (qc + 1)],
                            v_sb[kt][:, 65 * h:65 * h + 65],
                            pt[:, 512 * qc:512 * (qc + 1)],
                            start=(kt == 0), stop=(kt == 7))
                # drain o immediately (unnormalized) so the single o bank
                # frees; z row goes to the pair's z2 tile
                if jq not in pair_state:
                    rtf = main.tile([128, S], f32, tag="rtf", bufs=2,
                                    name=f"rtf{jq}")
                    zbc2 = main.tile([128, S], f32, tag="zb", bufs=2,
                                     name=f"zb{jq}")
                    pair_state[jq] = (rtf, zbc2)
                else:
                    rtf, zbc2 = pair_state[jq]
                o_cp = nc.scalar.copy if idx == 7 else nc.vector.tensor_copy
                # per-head Z -> 1/Z -> broadcast into the pair-wide zbc2 half
                # (z first: it gates the recip chain; rtf only gates the mul)
                zE = main.tile([1, S], f32, tag="z", bufs=2, name=f"z{h}")
                zi = main.tile([1, S], f32, tag="zi", bufs=2, name=f"zi{h}")
                o_cp(zE[0:1, :], o_ps[64:65, :])
                o_cp(rtf[row:row + 64, :], o_ps[0:64, :])
                nc.vector.reciprocal_approx_fast(zi[0:1, :], zE[0:1, :])
                if h % 2 == 0:
                    nc.gpsimd.partition_broadcast(zbc2[0:64, :], zi[0:1, :])
                else:
                    zscr = main.tile([64, S], f32, tag="zs", bufs=2,
                                     name=f"zs{jq}")
                    nc.gpsimd.partition_broadcast(zscr[0:64, :], zi[0:1, :])
                    nc.gpsimd.tensor_copy(zbc2[64:128, :], zscr[0:64, :])
                if idx % 2 == 1:  # pair complete -> normalize + s_in
                    nc.vector.tensor_mul(rtb[jq][:], rtf[:], zbc2[:])
                    # s_in^T[j] = rtb[j] + gate * ctokT[j]
                    nc.vector.scalar_tensor_tensor(
                        rtb[jq][:], ctokT[jq][:], g_sb[:, 0:1], rtb[jq][:],
                        MULT, ADD)


            emit_conv_cf(0, nc.vector.tensor_copy)
            emit_conv_cf(1, nc.scalar.copy)

        # ---- tail ----
        # stT rides the dead wkv mega-tag, s2d rides kT's
        stT_all = main.tile([128, 4 * S], bf16, tag="wkva", name="stT_all")
        stT = [stT_all[:, S * cc:S * (cc + 1)] for cc in range(4)]
        s2d = [main.tile([128, S], bf16, tag=f"kt{jj}", name=f"s2d{jj}")
               for jj in range(4)]
        with tc.tile_pool(name="psTail", bufs=1, space="PSUM") as psT:
            # stT[cc] = Wproj^T @ s_in^T; cc0 chains pre-start their
            # j=0..2 partials during the last pair's normalization (rtb[0..2]
            # are long since final); j=3 lands after s_in completes
            held = {}

            def emit_stT_pre(cc):
                for half in range(2):
                    acc = psT.tile([128, 512], f32, tag="st", bufs=4)
                    for j in range(3):
                        nc.tensor.matmul(
                            acc[:],
                            wp_sb[j][:, 128 * cc:128 * (cc + 1)],
                            rtb[j][:, 512 * half:512 * (half + 1)],
                            start=(j == 0), stop=False)
                    held[(cc, half)] = acc

            def emit_stT(cc):
                for half in range(2):
                    acc = held.pop((cc, half), None)
                    if acc is not None:
                        nc.tensor.matmul(
                            acc[:],
                            wp_sb[3][:, 128 * cc:128 * (cc + 1)],
                            rtb[3][:, 512 * half:512 * (half + 1)],
                            start=False, stop=True)
                    else:
                        acc = psT.tile([128, 512], f32, tag="st", bufs=4)
                        for j in range(4):
                            nc.tensor.matmul(
                                acc[:],
                                wp_sb[j][:, 128 * cc:128 * (cc + 1)],
                                rtb[j][:, 512 * half:512 * (half + 1)],
                                start=(j == 0), stop=(j == 3))
                    # ACT is idle once the exps finish; Identity+bias does
                    # the bproj add there instead of queueing on DVE behind
                    # the pair-3 normalization chain
                    nc.scalar.activation(
                        stT[cc][:, 512 * half:512 * (half + 1)], acc[:],
                        IDENT, bias=bpj_sb[cc][:, 0:1])

            # s2d repack: s2d[i, c + 512*par] = stT[c, 2i + par]; parity-major
            # so the pc=0 conv (and its output DMAs) can start early
            def emit_T_group(jj, par):
                # the 4 cc blocks of s2d[jj]'s `par` half into one PSUM tile,
                # then a single half-width copy
                tp = psT.tile([128, 512], bf16, tag="tp", bufs=2)
                for cc in range(4):
                    ev = stT[cc].rearrange("p (t two) -> p two t", two=2)
                    nc.tensor.transpose(
                        tp[:, 128 * cc:128 * (cc + 1)],
                        ev[:, par, 128 * jj:128 * (jj + 1)], ident[:])
                eng = nc.scalar.copy if jj % 2 == 0 else nc.vector.tensor_copy
                eng(s2d[jj][:, 512 * par:512 * (par + 1)], tp[:])

            def emit_conv_s(pc):
                for oc in range(4):
                    acc = psT.tile([128, 512], f32, tag="cva", bufs=2)
                    for jj in range(4):
                        nc.tensor.matmul(acc[:],
                                         wcs_sb[jj][:, 128 * oc:128 * (oc + 1)],
                                         s2d[jj][:, 512 * pc:512 * (pc + 1)],
                                         start=(jj == 0), stop=(jj == 3))
                    ost = main.tile([128, 512], f32, tag=f"ost{oc % 2}",
                                    bufs=2, name=f"ost{pc}_{oc}")
                    nc.vector.tensor_scalar_add(ost[:], acc[:],
                                                bcv_sb[oc][:, 0:1])
                    nc.sync.dma_start(
                        out_p[128 * oc:128 * (oc + 1),
                              512 * pc:512 * (pc + 1)], ost[:])

            emit_stT_pre(0)
            emit_stT_pre(1)
            for cc in range(4):
                emit_stT(cc)
            for jj in range(4):
                emit_T_group(jj, 0)
            emit_conv_s(0)
            for jj in range(4):
                emit_T_group(jj, 1)
            emit_conv_s(1)

    nc.compile()
    _CACHE["nc"] = nc
    return nc


def _shard_inputs(content_feat, components, pos_emb, Wq, Wkv, Wproj, bproj,
                  Wconv, bconv):
    import ml_dtypes

    bf = ml_dtypes.bfloat16
    f = np.float32
    pos2 = np.asarray(pos_emb, dtype=f).reshape(S, C)
    wq2 = np.asarray(Wq, dtype=f).astype(bf)
    wkv2 = np.asarray(Wkv, dtype=f).astype(bf)
    wp2 = np.asarray(Wproj, dtype=f).astype(bf)
    wcT = np.ascontiguousarray(np.asarray(Wconv, dtype=f).T).astype(bf)
    bcv = np.ascontiguousarray(np.asarray(bconv, dtype=f).reshape(C, 1))
    zeros = np.zeros((C, 1), dtype=f)
    bpj = np.ascontiguousarray(np.asarray(bproj, dtype=f).reshape(C, 1))
    in_maps = []
    for core in range(N_CORES):
        b, n = core // 4, core % 4
        first = n == 0
        in_maps.append({
            "cf": np.ascontiguousarray(
                np.asarray(content_feat[b], dtype=f).reshape(C, S)).astype(bf),
            "ctokT": np.ascontiguousarray(
                (np.asarray(content_feat[b], dtype=f).reshape(S, C)
                 + pos2).T).astype(bf),
            "compT": np.ascontiguousarray(
                (np.asarray(components[n, b], dtype=f).reshape(S, C)
                 + pos2).T).astype(bf),
            "wq": wq2,
            "wkv": wkv2,
            "wproj": wp2,
            "wconvT": np.ascontiguousarray(wcT[:C]),
            "wccsel": np.ascontiguousarray(wcT[C + 128 * n:C + 128 * (n + 1)]).T.copy(
                ).T if False else np.ascontiguousarray(
                wcT[C:, 128 * n:128 * (n + 1)]),
            "bprojT": bpj if first else zeros,
            "bconvT": bcv if first else zeros,
            "gate": np.full((128, 1), 1.0 if first else 0.0, dtype=f),
        })
    return in_maps


def _run(trace=False, **inputs):
    from concourse.bass_utils import run_bass_kernel_spmd

    nc = _build()
    in_maps = _shard_inputs(**inputs)
    res = run_bass_kernel_spmd(nc, in_maps, list(range(N_CORES)), trace=trace)
    outs = [np.asarray(res.results[i]["out_p"], dtype=np.float64)
            for i in range(N_CORES)]
    out = np.stack([outs[0] + outs[1] + outs[2] + outs[3],
                    outs[4] + outs[5] + outs[6] + outs[7]], axis=0)
    for core in range(N_CORES):
        b, n = core // 4, core % 4
        out[b, 128 * n:128 * (n + 1), :] += np.asarray(
            res.results[core]["out_cf"], dtype=np.float64)
    return out.reshape(B, C, H, W).astype(np.float32), res


def kernel(**inputs):
    out, _ = _run(trace=False, **inputs)
    return out


# revision 56
# speedup vs baseline: 1.0004x; 1.0004x over previous
"""Trainium2 Bass kernel for nn_Attention_54391465836966.

Math (per batch b):
  ctok = content_feat[b].raw_reshape(S,C) + pos         # [1024, 512]
  comp_tok[n] = components[n,b].raw_reshape(S,C) + pos
  q = ctok @ Wq ; k[n],v[n] = comp_tok[n] @ Wkv (split)
  per head h, comp n: P = exp(scale * q_h k_h^T); o_nh = (P @ v_nh) / rowsum(P)
  result = sum_n o_n ; s = (result + ctok) @ Wproj + bproj
  out = Wconv^T[C:] @ cf2d + Wconv^T[:C] @ s2d + bconv
    (s2d = raw reshape of the token-major s buffer; cf2d = content_feat[b])

Sharding: 8 cores <- (b, n) pairs; b = core//4, n = core%4.  Everything after
`result` is affine in the component partial, so each core applies the linear
tail to its own o_n (constant terms gated to the n==0 core via zeroed
per-core inputs) and the host sums the four partial outputs per batch.  The
cf half of the conv depends only on b, so it is additionally sharded over n:
each core computes just its own 128-row output-channel chunk (out_cf) and
the host scatter-adds it into the gathered result.

Implementation notes:
- The reference's token matrices are RAW reshapes of [C,H,W] buffers (the
  same even/odd interleave as the final s->s2d reshape), so the host ships
  (raw_reshape(x) + pos).T directly (bf16); nothing on-chip ever transposes
  tokens.  Wconv is host-transposed as well.
- All matmul operands are bf16 (fp32 PSUM accumulate).  DMA volume is half
  of fp32, and transposes run at 1.0 cycles/row.
- Attention is ACT-bound (64 exps of [128,1024] ~= 1 us each), so the
  kT/qT matmuls of later head-pairs are interleaved INTO the attention
  stream to soak up spare PE cycles.  PSUM budget makes this possible:
  o is single-buffered and drained immediately with UNnormalized copies;
  normalization (reciprocal_approx_fast + partition_broadcast + one
  pair-wide [128,S] multiply) happens in SBUF afterwards
  -> mm(2) + sc(2x2) + o(2) = 8 banks.  Each input tensor is a single
  merged DMA (per-DMA cost is ~650ns serial SP.SEQ + ~625ns HWDGE + 900ns
  sem latency), and dep-free warmup transposes spin the PE clock up past
  its p-state ramp before the first real matmuls.
- Projection is computed transposed (stT = Wproj^T @ s_in^T) with head
  pairs packed into 128-partition tiles (full-K matmuls), bias via
  per-partition tensor_scalar on the PSUM drain.  The s->s2d raw-reshape
  is 32 PE transposes of stride-2 column slices, interleaved parity-major
  with the conv so output DMAs start early.  The cf half of the conv is
  emitted at the head of the tail to cover the last pair's normalization
  chain.
"""
import sys

sys.path.insert(0, "/opt/trn_rl_repo")

import numpy as np

N_CORES = 8
B, C, H, W = 2, 512, 32, 32
S = H * W  # 1024
NH, HD = 8, 64
SCALE = HD ** -0.5

_CACHE = {}


def _build():
    if "nc" in _CACHE:
        return _CACHE["nc"]
    from contextlib import ExitStack

    import concourse.bacc as bacc
    import concourse.mybir as mybir
    import concourse.tile as tile
    from concourse.masks import make_identity

    f32 = mybir.dt.float32
    f32r = mybir.dt.float32r
    bf16 = mybir.dt.bfloat16
    EXP = mybir.ActivationFunctionType.Exp
    IDENT = mybir.ActivationFunctionType.Identity
    MULT = mybir.AluOpType.mult
    ADD = mybir.AluOpType.add

    nc = bacc.Bacc("TRN2", target_bir_lowering=False, debug=False,
                   num_devices=N_CORES)

    din = lambda n, s, dt: nc.dram_tensor(n, s, dt, kind="ExternalInput").ap()
    cf = din("cf", [C, S], bf16)         # content_feat[b] raw [C,S] (conv only)
    ctokTd = din("ctokT", [C, S], bf16)  # (content_tok + pos).T, host-prepped
    compTd = din("compT", [C, S], bf16)  # (comp_tok + pos).T, host-prepped
    wq = din("wq", [C, C], bf16)
    wkv = din("wkv", [C, 2 * C], bf16)   # cols 0:C -> K, C:2C -> V
    wproj = din("wproj", [C, C], bf16)
    wconvT = din("wconvT", [C, C], bf16)      # Wconv.T rows :C (s half)
    wccsel = din("wccsel", [C, 128], bf16)    # Wconv.T[C:, 128n:128n+128]
    bprojT = din("bprojT", [C, 1], f32)       # zeroed n>0
    bconvT = din("bconvT", [C, 1], f32)       # zeroed n>0
    gate = din("gate", [128, 1], f32)         # 1.0 on n==0 cores else 0.0
    out_p = nc.dram_tensor("out_p", [C, S], f32, kind="ExternalOutput").ap()
    out_cf = nc.dram_tensor("out_cf", [128, S], f32,
                            kind="ExternalOutput").ap()

    with tile.TileContext(nc) as tc, ExitStack() as ctx:
        main = ctx.enter_context(tc.tile_pool(name="main", bufs=1))

        # ---- constants ----
        ident32 = main.tile([128, 128], f32, tag="id32")
        make_identity(nc, ident32[:])
        ident = main.tile([128, 128], bf16, tag="ident")
        nc.vector.tensor_copy(ident[:], ident32[:])
        ones_bf = main.tile([128, 8], bf16, tag="ones")
        nc.gpsimd.memset(ones_bf[:], 1.0)
        g_sb = main.tile([128, 1], f32, tag="g")

        # ---- persistent SBUF tiles (one wide tile per DRAM tensor so each
        # input is a SINGLE DMA: per-DMA cost is ~650ns serial SP.SEQ +
        # ~625ns HWDGE + 900ns sem latency, so fewer/bigger wins) ----
        wkv_all = main.tile([128, 4 * 2 * C], bf16, tag="wkva", name="wkva")
        wq_all = main.tile([128, 4 * C], bf16, tag="wqa", name="wqa")
        comp_all = main.tile([128, 4 * S], bf16, tag="cma", name="cma")
        cf_all = main.tile([128, 4 * S], bf16, tag="cfa", name="cfa")
        ctok_all = main.tile([128, 4 * S], bf16, tag="cta", name="cta")
        wc_all = main.tile([128, 4 * C], bf16, tag="wca", name="wca")
        wcsel_all = main.tile([128, 4 * 128], bf16, tag="wcsa", name="wcsa")
        wp_all = main.tile([128, 4 * C], bf16, tag="wpa", name="wpa")
        bpj_all = main.tile([128, 4], f32, tag="bpja", name="bpja")
        bcv_all = main.tile([128, 4], f32, tag="bcva", name="bcva")
        # chunk views (k-th 128-row block of the [C,*] DRAM tensor)
        wkv_sb = [wkv_all[:, 2 * C * k:2 * C * (k + 1)] for k in range(4)]
        wq_sb = [wq_all[:, C * k:C * (k + 1)] for k in range(4)]
        comp_sb = [comp_all[:, S * k:S * (k + 1)] for k in range(4)]
        cf_sb = [cf_all[:, S * k:S * (k + 1)] for k in range(4)]
        ctokT = [ctok_all[:, S * k:S * (k + 1)] for k in range(4)]
        wcs_sb = [wc_all[:, C * k:C * (k + 1)] for k in range(4)]
        wcc_sb = [wcsel_all[:, 128 * k:128 * (k + 1)] for k in range(4)]
        wp_sb = [wp_all[:, C * k:C * (k + 1)] for k in range(4)]
        bcv_sb = [bcv_all[:, i:i + 1] for i in range(4)]
        bpj_sb = [bpj_all[:, i:i + 1] for i in range(4)]
        kT = [main.tile([128, S], bf16, tag=f"kt{j}", name=f"kt{j}")
              for j in range(4)]
        qT = [main.tile([128, S], bf16, tag=f"qt{j}", name=f"qt{j}")
              for j in range(4)]
        v_sb = [main.tile([128, 8 * 65], bf16, tag=f"v{t}", name=f"v{t}")
                for t in range(8)]
        outcf_sb = main.tile([128, S], f32, tag="ocf", name="ocf")
        # rtb[j]: normalized bf16 head pair (2j, 2j+1), later s_in^T chunk j
        rtb = [main.tile([128, S], bf16, tag=f"rt{j}", name=f"rt{j}")
               for j in range(4)]

        # ---- DMA emission order: attention-critical first ----
        def dma_merged(dst_tile, src_ap, k):
            src3 = src_ap.rearrange("(k p) s -> p k s", k=k)
            dst3 = dst_tile[:].rearrange("p (k s) -> p k s", k=k)
            nc.sync.dma_start(dst3[:, :, :], src3[:, :, :])

        def dma_merged_part(dst_tile, src_ap, k, lo, hi, inner):
            src3 = src_ap.rearrange("(k p) s -> p k s", k=k)
            dst3 = dst_tile[:].rearrange("p (k s) -> p k s", k=k)
            nc.sync.dma_start(dst3[:, lo:hi, 0:inner], src3[:, lo:hi, 0:inner])

        dma_merged_part(comp_all, compTd, 4, 0, 2, S)
        dma_merged_part(wkv_all, wkv, 4, 0, 2, 2 * C)
        dma_merged_part(comp_all, compTd, 4, 2, 4, S)
        dma_merged_part(wkv_all, wkv, 4, 2, 4, 2 * C)
        dma_merged(wq_all, wq, 4)
        dma_merged(ctok_all, ctokTd, 4)
        dma_merged(cf_all, cf, 4)
        dma_merged(wcsel_all, wccsel, 4)
        nc.sync.dma_start(g_sb[:], gate[:])
        dma_merged(bcv_all, bconvT, 4)
        dma_merged(bpj_all, bprojT, 4)
        dma_merged(wp_all, wproj, 4)
        dma_merged(wc_all, wconvT, 4)

        warm_src = main.tile([128, 128], bf16, tag="warm", name="warm")
        nc.gpsimd.memset(warm_src[:], 0.25)

        # one PSUM pool spans startup + attention: mm(1x2) + sc(2x2) + o(2x1)
        # = 8 banks.  The mm tag stays live through attention so the later
        # head-pairs' kT/qT matmuls can interleave into ACT-bound stretches.
        with tc.tile_pool(name="psMain", bufs=1, space="PSUM") as ps:

            def emit_kq(j, dst, wsrc, act, cp_eng):
                for t in range(2):
                    acc = ps.tile([128, 512], f32, tag="mm", bufs=2)
                    for k in range(4):
                        nc.tensor.matmul(acc[:],
                                         wsrc[k][:, 128 * j:128 * (j + 1)],
                                         act[k][:, 512 * t:512 * (t + 1)],
                                         start=(k == 0), stop=(k == 3))
                    cp_eng(dst[j][:, 512 * t:512 * (t + 1)], acc[:])

            for w in range(36):
                wtp = ps.tile([128, 128], bf16, tag="mm", bufs=2)
                nc.tensor.transpose(wtp[:], warm_src[:], ident[:])

            emit_kq(0, kT, wkv_sb, comp_sb, nc.scalar.copy)

            # v (lhsT of the o matmuls).  v0-3 run while the ctok DMA is in
            # flight; qT[0] follows, then v4-7 interleave with head 0's first
            # scores so the ACT exp stream starts as early as possible.
            h0_sc = []

            def emit_v(t):
                nc.vector.tensor_copy(
                    v_sb[t][:].rearrange("p (h e) -> p h e", h=8)[:, :, 64:65],
                    ones_bf[:, 0:8].rearrange("p (h o) -> p h o", o=1))
                acc = ps.tile([128, 512], f32, tag="mm", bufs=2)
                for k in range(4):
                    nc.tensor.matmul(acc[:], comp_sb[k][:, 128 * t:128 * (t + 1)],
                                     wkv_sb[k][:, C:2 * C],
                                     start=(k == 0), stop=(k == 3))
                nc.vector.tensor_copy(
                    v_sb[t][:].rearrange("p (h e) -> p h e", h=8)[:, :, 0:64],
                    acc[:].rearrange("p (h d) -> p h d", h=8))

            def emit_h0_sc(t):
                sc = ps.tile([128, S], f32, tag="sc", bufs=2)
                for qc in range(2):
                    nc.tensor.matmul(
                        sc[:, 512 * qc:512 * (qc + 1)],
                        kT[0][0:64, 128 * t:128 * (t + 1)],
                        qT[0][0:64, 512 * qc:512 * (qc + 1)],
                        start=True, stop=True)
                pt = main.tile([128, S], bf16, tag=f"pt{t}", name=f"pt0_{t}")
                nc.scalar.activation(pt[:], sc[:], EXP, scale=SCALE)
                h0_sc.append((sc, pt))

            for t in range(4):
                emit_v(t)
            emit_kq(0, qT, wq_sb, ctokT, nc.scalar.copy)
            for t in range(4):
                emit_h0_sc(t)
                emit_v(4 + t)
            for t in range(4, 8):
                emit_h0_sc(t)

            # ---- attention ----
            # head order: pairs 0..3; within the LAST pair the odd head goes
            # first so the closing normalization chain is one op shorter.
            heads = [0, 1, 2, 3, 4, 5, 7, 6]
            sc_prev = None
            pre_sc1 = [None]
            pair_state = {}

            def emit_conv_cf(pc, cp_eng, pool=None, tag="mm"):
                acc = (pool or ps).tile([128, 512], f32, tag=tag, bufs=2)
                for k2 in range(4):
                    nc.tensor.matmul(acc[:], wcc_sb[k2][:],
                                     cf_sb[k2][:, 512 * pc:512 * (pc + 1)],
                                     start=(k2 == 0), stop=(k2 == 3))
                cp_eng(outcf_sb[:, 512 * pc:512 * (pc + 1)], acc[:])
                nc.sync.dma_start(
                    out_cf[:, 512 * pc:512 * (pc + 1)],
                    outcf_sb[:, 512 * pc:512 * (pc + 1)])
            for idx, h in enumerate(heads):
                jq, row = h // 2, 64 * (h % 2)
                o_ps = ps.tile([65, S], f32, tag="o", bufs=1)
                scs = []
                for kt in range(8):
                    if idx == 1 and kt == 5:
                        emit_kq(1, kT, wkv_sb, comp_sb, nc.vector.tensor_copy)
                    elif idx == 1 and kt == 6:
                        emit_kq(1, qT, wq_sb, ctokT, nc.vector.tensor_copy)
                    elif kt == 6 and 2 <= idx < 6 and jq < 3:
                        if idx % 2 == 0:
                            emit_kq(jq + 1, kT, wkv_sb, comp_sb,
                                    nc.vector.tensor_copy)
                        else:
                            emit_kq(jq + 1, qT, wq_sb, ctokT,
                                    nc.vector.tensor_copy)
                    if idx == 0:
                        scs.append(None)
                        continue
                    if kt == 1 and idx == 1 and pre_sc1[0] is not None:
                        sc = pre_sc1[0]
                    elif kt > 0:
                        sc = ps.tile([128, S], f32, tag="sc", bufs=2)
                        for qc in range(2):
                            nc.tensor.matmul(
                                sc[:, 512 * qc:512 * (qc + 1)],
                                kT[jq][row:row + 64, 128 * kt:128 * (kt + 1)],
                                qT[jq][row:row + 64, 512 * qc:512 * (qc + 1)],
                                start=True, stop=True)
                    else:
                        sc = sc_prev
                    scs.append(sc)
                for kt in range(8):
                    if idx == 0 and kt in (0, 1):
                        # pre-emit next head's kt=0/1 scores (h0's own scores
                        # were emitted interleaved with v above, so its o
                        # matmuls leave ACT hungry at the boundary)
                        h2 = heads[1]
                        jq2, row2 = h2 // 2, 64 * (h2 % 2)
                        scn = ps.tile([128, S], f32, tag="sc", bufs=2)
                        for qc in range(2):
                            nc.tensor.matmul(
                                scn[:, 512 * qc:512 * (qc + 1)],
                                kT[jq2][row2:row2 + 64,
                                        128 * kt:128 * (kt + 1)],
                                qT[jq2][row2:row2 + 64,
                                        512 * qc:512 * (qc + 1)],
                                start=True, stop=True)
                        if kt == 0:
                            sc_prev = scn
                        else:
                            pre_sc1[0] = scn
                    if idx == 0:
                        pt = h0_sc[kt][1]
                        for qc in range(2):
                            nc.tensor.matmul(
                                o_ps[:, 512 * qc:512 * (qc + 1)],
                                v_sb[kt][:, 65 * h:65 * h + 65],
                                pt[:, 512 * qc:512 * (qc + 1)],
                                start=(kt == 0), stop=(kt == 7))
                        continue
                    if idx < NH - 1 and kt == 7:
                        # pre-emit next head's kt=0 scores (keeps ACT fed
                        # across the head boundary)
                        h2 = heads[idx + 1]
                        jq2, row2 = h2 // 2, 64 * (h2 % 2)
                        sc_prev = ps.tile([128, S], f32, tag="sc", bufs=2)
                        for qc in range(2):
                            nc.tensor.matmul(
                                sc_prev[:, 512 * qc:512 * (qc + 1)],
                                kT[jq2][row2:row2 + 64, 0:128],
                                qT[jq2][row2:row2 + 64,
                                        512 * qc:512 * (qc + 1)],
                                start=True, stop=True)
                    pt = main.tile([128, S], bf16, tag=f"pt{kt}",
                                   name=f"pt{h}_{kt}")
                    nc.scalar.activation(pt[:], scs[kt][:], EXP, scale=SCALE)
                    for qc in range(2):
                        nc.tensor.matmul(
                            o_ps[:, 512 * qc:512 * (qc + 1)],
                            v_sb[kt][:, 65 * h:65 * h + 65],
                            pt[:, 512 * qc:512 * (qc + 1)],
                            start=(kt == 0), stop=(kt == 7))
                # drain o immediately (unnormalized) so the single o bank
                # frees; z row goes to the pair's z2 tile
                if jq not in pair_state:
                    rtf = main.tile([128, S], f32, tag="rtf", bufs=2,
                                    name=f"rtf{jq}")
                    zbc2 = main.tile([128, S], f32, tag="zb", bufs=2,
                                     name=f"zb{jq}")
                    pair_state[jq] = (rtf, zbc2)
                else:
                    rtf, zbc2 = pair_state[jq]
                o_cp = nc.scalar.copy if idx == 7 else nc.vector.tensor_copy
                # per-head Z -> 1/Z -> broadcast into the pair-wide zbc2 half
                # (z first: it gates the recip chain; rtf only gates the mul)
                zE = main.tile([1, S], f32, tag="z", bufs=2, name=f"z{h}")
                zi = main.tile([1, S], f32, tag="zi", bufs=2, name=f"zi{h}")
                o_cp(zE[0:1, :], o_ps[64:65, :])
                o_cp(rtf[row:row + 64, :], o_ps[0:64, :])
                nc.vector.reciprocal_approx_fast(zi[0:1, :], zE[0:1, :])
                if h % 2 == 0:
                    nc.gpsimd.partition_broadcast(zbc2[0:64, :], zi[0:1, :])
                else:
                    zscr = main.tile([64, S], f32, tag="zs", bufs=2,
                                     name=f"zs{jq}")
                    nc.gpsimd.partition_broadcast(zscr[0:64, :], zi[0:1, :])
                    nc.gpsimd.tensor_copy(zbc2[64:128, :], zscr[0:64, :])
                if idx % 2 == 1:  # pair complete -> normalize + s_in
                    nc.vector.tensor_mul(rtb[jq][:], rtf[:], zbc2[:])
                    # s_in^T[j] = rtb[j] + gate * ctokT[j]
                    nc.vector.scalar_tensor_tensor(
                        rtb[jq][:], ctokT[jq][:], g_sb[:, 0:1], rtb[jq][:],
                        MULT, ADD)


            emit_conv_cf(0, nc.vector.tensor_copy)
            emit_conv_cf(1, nc.scalar.copy)

        # ---- tail ----
        # stT rides the dead wkv mega-tag, s2d rides kT's
        stT_all = main.tile([128, 4 * S], bf16, tag="wkva", name="stT_all")
        stT = [stT_all[:, S * cc:S * (cc + 1)] for cc in range(4)]
        s2d = [main.tile([128, S], bf16, tag=f"kt{jj}", name=f"s2d{jj}")
               for jj in range(4)]
        with tc.tile_pool(name="psTail", bufs=1, space="PSUM") as psT:
            # stT[cc] = Wproj^T @ s_in^T; cc0 chains pre-start their
            # j=0..2 partials during the last pair's normalization (rtb[0..2]
            # are long since final); j=3 lands after s_in completes
            held = {}

            def emit_stT_pre(cc):
                for half in range(2):
                    acc = psT.tile([128, 512], f32, tag="st", bufs=4)
                    for j in range(3):
                        nc.tensor.matmul(
                            acc[:],
                            wp_sb[j][:, 128 * cc:128 * (cc + 1)],
                            rtb[j][:, 512 * half:512 * (half + 1)],
                            start=(j == 0), stop=False)
                    held[(cc, half)] = acc

            def emit_stT(cc):
                for half in range(2):
                    acc = held.pop((cc, half), None)
                    if acc is not None:
                        nc.tensor.matmul(
                            acc[:],
                            wp_sb[3][:, 128 * cc:128 * (cc + 1)],
                            rtb[3][:, 512 * half:512 * (half + 1)],
                            start=False, stop=True)
                    else:
                        acc = psT.tile([128, 512], f32, tag="st", bufs=4)
                        for j in range(4):
                            nc.tensor.matmul(
                                acc[:],
                                wp_sb[j][:, 128 * cc:128 * (cc + 1)],
                                rtb[j][:, 512 * half:512 * (half + 1)],
                                start=(j == 0), stop=(j == 3))
                    # ACT is idle once the exps finish; Identity+bias does
                    # the bproj add there instead of queueing on DVE behind
                    # the pair-3 normalization chain
                    nc.scalar.activation(
                        stT[cc][:, 512 * half:512 * (half + 1)], acc[:],
                        IDENT, bias=bpj_sb[cc][:, 0:1])

            # s2d repack: s2d[i, c + 512*par] = stT[c, 2i + par]; parity-major
            # so the pc=0 conv (and its output DMAs) can start early
            def emit_T_group(jj, par):
                # the 4 cc blocks of s2d[jj]'s `par` half into one PSUM tile,
                # then a single half-width copy
                tp = psT.tile([128, 512], bf16, tag="tp", bufs=2)
                for cc in range(4):
                    ev = stT[cc].rearrange("p (t two) -> p two t", two=2)
                    nc.tensor.transpose(
                        tp[:, 128 * cc:128 * (cc + 1)],
                        ev[:, par, 128 * jj:128 * (jj + 1)], ident[:])
                eng = nc.scalar.copy if jj % 2 == 0 else nc.vector.tensor_copy
                eng(s2d[jj][:, 512 * par:512 * (par + 1)], tp[:])

            def emit_conv_s(pc):
                for oc in range(4):
                    acc = psT.tile([128, 512], f32, tag="cva", bufs=2)
                    for jj in range(4):
                        nc.tensor.matmul(acc[:],
                                         wcs_sb[jj][:, 128 * oc:128 * (oc + 1)],
                                         s2d[jj][:, 512 * pc:512 * (pc + 1)],
                                         start=(jj == 0), stop=(jj == 3))
                    ost = main.tile([128, 512], f32, tag=f"ost{oc % 2}",
                                    bufs=2, name=f"ost{pc}_{oc}")
                    nc.vector.tensor_scalar_add(ost[:], acc[:],
                                                bcv_sb[oc][:, 0:1])
                    nc.sync.dma_start(
                        out_p[128 * oc:128 * (oc + 1),
                              512 * pc:512 * (pc + 1)], ost[:])

            emit_stT_pre(0)
            emit_stT_pre(1)
            for cc in range(4):
                emit_stT(cc)
            for jj in range(4):
                emit_T_group(jj, 0)
            emit_conv_s(0)
            for jj in range(4):
                emit_T_group(jj, 1)
            emit_conv_s(1)

    nc.compile()
    _CACHE["nc"] = nc
    return nc


def _shard_inputs(content_feat, components, pos_emb, Wq, Wkv, Wproj, bproj,
                  Wconv, bconv):
    import ml_dtypes

    bf = ml_dtypes.bfloat16
    f = np.float32
    pos2 = np.asarray(pos_emb, dtype=f).reshape(S, C)
    wq2 = np.asarray(Wq, dtype=f).astype(bf)
    wkv2 = np.asarray(Wkv, dtype=f).astype(bf)
    wp2 = np.asarray(Wproj, dtype=f).astype(bf)
    wcT = np.ascontiguousarray(np.asarray(Wconv, dtype=f).T).astype(bf)
    bcv = np.ascontiguousarray(np.asarray(bconv, dtype=f).reshape(C, 1))
    zeros = np.zeros((C, 1), dtype=f)
    bpj = np.ascontiguousarray(np.asarray(bproj, dtype=f).reshape(C, 1))
    in_maps = []
    for core in range(N_CORES):
        b, n = core // 4, core % 4
        first = n == 0
        in_maps.append({
            "cf": np.ascontiguousarray(
                np.asarray(content_feat[b], dtype=f).reshape(C, S)).astype(bf),
            "ctokT": np.ascontiguousarray(
                (np.asarray(content_feat[b], dtype=f).reshape(S, C)
                 + pos2).T).astype(bf),
            "compT": np.ascontiguousarray(
                (np.asarray(components[n, b], dtype=f).reshape(S, C)
                 + pos2).T).astype(bf),
            "wq": wq2,
            "wkv": wkv2,
            "wproj": wp2,
            "wconvT": np.ascontiguousarray(wcT[:C]),
            "wccsel": np.ascontiguousarray(wcT[C + 128 * n:C + 128 * (n + 1)]).T.copy(
                ).T if False else np.ascontiguousarray(
                wcT[C:, 128 * n:128 * (n + 1)]),
            "bprojT": bpj if first else zeros,
            "bconvT": bcv if first else zeros,
            "gate": np.full((128, 1), 1.0 if first else 0.0, dtype=f),
        })
    return in_maps


def _run(trace=False, **inputs):
    from concourse.bass_utils import run_bass_kernel_spmd

    nc = _build()
    in_maps = _shard_inputs(**inputs)
    res = run_bass_kernel_spmd(nc, in_maps, list(range(N_CORES)), trace=trace)
    outs = [np.asarray(res.results[i]["out_p"], dtype=np.float64)
            for i in range(N_CORES)]
    out = np.stack([outs[0] + outs[1] + outs[2] + outs[3],
                    outs[4] + outs[5] + outs[6] + outs[7]], axis=0)
    for core in range(N_CORES):
        b, n = core // 4, core % 4
        out[b, 128 * n:128 * (n + 1), :] += np.asarray(
            res.results[core]["out_cf"], dtype=np.float64)
    return out.reshape(B, C, H, W).astype(np.float32), res


def kernel(**inputs):
    out, _ = _run(trace=False, **inputs)
    return out
